# revision 3
# baseline (speedup 1.0000x reference)
"""Trainium2 Bass kernel for nn_ExchangeBlock (gnn_message_passing), v2.

Data-parallel over edges: each of the 8 cores processes E/8 = 16384 edges in
32 tiles of 512. Per tile:
  - indirect-DMA gather of bf16 node rows for src/dst (radial scalars r and
    EMBC*dist are precomputed per edge on the host and shipped directly);
  - Bessel embedding via range-reduced Sin on the Act engine (the whole tile
    loop stays inside the one `silu_and_others` act table: no table reloads);
  - tensor product computed EDGE-MAJOR: the outer products e0/e1/e2 are pure
    broadcast-AP elementwise ops on DVE (bf16, 2x mode), then PE-transposed
    (bf16, 1 cycle/row) to uv-major for the W-contraction, m-outer with per-m
    evacuation so LN stats overlap the df MLP;
  - LayerNorm: all-ones 128x128 stat matmuls (broadcast built in) + DVE
    Newton-rsqrt;
  - df/mix MLPs as bf16 matmuls with all weights SBUF-resident; the final
    mo_w contraction runs as DVE scalar_tensor_tensor + one ones-matmul.
Per-engine budget (cost model, per tile): PE ~44us (90% busy), DVE ~22us,
Act ~21us; next-tile gather/product front-end is priority-hoisted over the
current tile's MLP tail.
"""
import os
import sys

sys.path.insert(0, "/opt/trn_rl_repo")

import math
import numpy as np

L0, L1, L2 = 32, 16, 8
NS, NB = 512, 256
CUT = 7.0
N, E, G = 16384, 131072, 16
FEAT = L0 + 3 * L1 + 5 * L2  # 120
NCORES = 8
EC = E // NCORES
BLK = 128
ET = 512
NBLK = ET // BLK
FAN = math.sqrt(float(L0 * L0 + L1 * L1 + L2 * L2))
EMBC = math.sqrt(2.0 / CUT)
O1 = L0
O2 = L0 + 3 * L1

_cache = {}


def _build(mode, ntiles, reps=1):
    import concourse.bacc as bacc
    import concourse.bass as bass
    import concourse.mybir as mybir
    import concourse.tile as tile

    f32 = mybir.dt.float32
    bf16 = mybir.dt.bfloat16
    i32 = mybir.dt.int32
    AF = mybir.ActivationFunctionType
    OP = mybir.AluOpType

    nc = bacc.Bacc(None)

    # ---------------- DRAM tensors ----------------
    nodesB = nc.dram_tensor("nodesB", [N, FEAT], bf16, kind="ExternalInput")
    srcidx = nc.dram_tensor("srcidx", [ntiles, BLK, NBLK], i32, kind="ExternalInput")
    dstidx = nc.dram_tensor("dstidx", [ntiles, BLK, NBLK], i32, kind="ExternalInput")
    rdd = nc.dram_tensor("rdd", [ntiles, BLK, NBLK, 2], f32, kind="ExternalInput")

    w0d = nc.dram_tensor("w0d", [1024, NS], bf16, kind="ExternalInput")
    w1d = nc.dram_tensor("w1d", [256, NS], bf16, kind="ExternalInput")
    w2d = nc.dram_tensor("w2d", [64, NS], bf16, kind="ExternalInput")
    dfw1d = nc.dram_tensor("dfw1d", [256, 1024], bf16, kind="ExternalInput")
    dfw2d = nc.dram_tensor("dfw2d", [1024, NS], bf16, kind="ExternalInput")
    miw1d = nc.dram_tensor("miw1d", [NS, 1024], bf16, kind="ExternalInput")
    miw2d = nc.dram_tensor("miw2d", [1024, 1024], bf16, kind="ExternalInput")
    mowd = nc.dram_tensor("mowd", [1024, 1], f32, kind="ExternalInput")
    bdf1 = nc.dram_tensor("bdf1", [BLK, 8], f32, kind="ExternalInput")
    bdf2 = nc.dram_tensor("bdf2", [BLK, 4], f32, kind="ExternalInput")
    bmi1 = nc.dram_tensor("bmi1", [BLK, 8], f32, kind="ExternalInput")
    bmi2 = nc.dram_tensor("bmi2", [BLK, 8], f32, kind="ExternalInput")
    bmo = nc.dram_tensor("bmo", [1, 1], f32, kind="ExternalInput")
    cnd = nc.dram_tensor("cnd", [BLK, NB], f32, kind="ExternalInput")
    identd = nc.dram_tensor("identd", [BLK, BLK], bf16, kind="ExternalInput")
    onesd = nc.dram_tensor("onesd", [BLK, BLK], bf16, kind="ExternalInput")

    outd = nc.dram_tensor("out", [ntiles, 1, ET], f32, kind="ExternalOutput")

    TWO_PI = 2.0 * math.pi
    sin_bias = -math.pi if mode == "sim" else 0.0
    sim_neg = -1.0 if mode == "sim" else 1.0

    with tile.TileContext(nc) as tc:
        with (
            tc.tile_pool(name="const", bufs=1) as cp,
            tc.tile_pool(name="gat", bufs=2) as gp,
            tc.tile_pool(name="emb", bufs=1) as ep,
            tc.tile_pool(name="embs", bufs=2) as eps_p,
            tc.tile_pool(name="eprod", bufs=2) as pp,
            tc.tile_pool(name="etr", bufs=1) as xp,
            tc.tile_pool(name="mid", bufs=1) as mp,
            tc.tile_pool(name="sml", bufs=2) as sp,
            tc.tile_pool(name="act", bufs=1) as hp,
            tc.tile_pool(name="pstr", bufs=2, space="PSUM") as pst,
            tc.tile_pool(name="psph", bufs=3, space="PSUM") as psh,
            tc.tile_pool(name="pspo", bufs=1, space="PSUM") as pso,
            tc.tile_pool(name="psmix", bufs=2, space="PSUM") as psm,
        ):
            # ---------------- constants ----------------
            w0_t = cp.tile([BLK, 8, NS], bf16)
            for c in range(8):
                nc.sync.dma_start(w0_t[:, c, :], w0d[c * BLK:(c + 1) * BLK, :])
            w1_t = cp.tile([BLK, 2, NS], bf16)
            for c in range(2):
                nc.sync.dma_start(w1_t[:, c, :], w1d[c * BLK:(c + 1) * BLK, :])
            w2_t = cp.tile([64, NS], bf16)
            nc.sync.dma_start(w2_t[:], w2d[:])
            dfw1_t = cp.tile([BLK, 2, 1024], bf16)
            for c in range(2):
                nc.sync.dma_start(dfw1_t[:, c, :], dfw1d[c * BLK:(c + 1) * BLK, :])
            dfw2_t = cp.tile([BLK, 8, NS], bf16)
            for c in range(8):
                nc.sync.dma_start(dfw2_t[:, c, :], dfw2d[c * BLK:(c + 1) * BLK, :])
            miw1_t = cp.tile([BLK, 4, 1024], bf16)
            for c in range(4):
                nc.sync.dma_start(miw1_t[:, c, :], miw1d[c * BLK:(c + 1) * BLK, :])
            miw2_t = cp.tile([BLK, 8, 1024], bf16)
            for c in range(8):
                nc.sync.dma_start(miw2_t[:, c, :], miw2d[c * BLK:(c + 1) * BLK, :])
            mowf_t = cp.tile([BLK, 8], f32)
            nc.sync.dma_start(mowf_t[:], mowd[:].rearrange("(c p) one -> p (c one)", p=BLK))
            ones32_t = cp.tile([BLK, 1], f32)
            nc.gpsimd.memset(ones32_t[:], 1.0)
            bdf1_t = cp.tile([BLK, 8], f32)
            nc.sync.dma_start(bdf1_t[:], bdf1[:])
            bdf2_t = cp.tile([BLK, 4], f32)
            nc.sync.dma_start(bdf2_t[:], bdf2[:])
            bmi1_t = cp.tile([BLK, 8], f32)
            nc.sync.dma_start(bmi1_t[:], bmi1[:])
            bmi2_t = cp.tile([BLK, 8], f32)
            nc.sync.dma_start(bmi2_t[:], bmi2[:])
            bmo_t = cp.tile([1, 1], f32)
            nc.sync.dma_start(bmo_t[:], bmo[:])
            cn_t = cp.tile([BLK, NB], f32)
            nc.sync.dma_start(cn_t[:], cnd[:])
            id_t = cp.tile([BLK, BLK], bf16)
            nc.sync.dma_start(id_t[:], identd[:])
            ones_t = cp.tile([BLK, BLK], bf16)
            nc.sync.dma_start(ones_t[:], onesd[:])
            sinb_t = cp.tile([BLK, 1], f32)
            nc.gpsimd.memset(sinb_t[:], sin_bias)
            eps_t = cp.tile([BLK, 1], f32)
            nc.gpsimd.memset(eps_t[:], 1e-5)

            def silu_to(dst, ps, bias_ap):
                if mode == "sim":
                    sg = sp.tile([BLK, ET], f32, tag="sg")
                    nc.scalar.activation(sg[:], ps, AF.Sigmoid, bias=bias_ap, scale=1.0)
                    pre = sp.tile([BLK, ET], f32, tag="pre")
                    nc.vector.tensor_scalar(out=pre[:], in0=ps, scalar1=bias_ap,
                                            scalar2=None, op0=OP.add)
                    nc.vector.tensor_tensor(out=dst, in0=sg[:], in1=pre[:], op=OP.mult)
                else:
                    nc.scalar.activation(dst, ps, AF.Silu, bias=bias_ap, scale=1.0)

            for t in [tt for _ in range(reps) for tt in range(ntiles)]:
                # ============ index / scalar loads + gathers ============
                # high_priority: let tile t's gather/emb/TP-product front-end
                # schedule ahead of tile t-1's MLP tail so DMA/DVE/Pool overlap PE
                hctx = tc.high_priority(100)
                hctx.__enter__()
                sidx = gp.tile([BLK, NBLK], i32, tag="sidx")
                didx = gp.tile([BLK, NBLK], i32, tag="didx")
                rdt = gp.tile([BLK, NBLK, 2], f32, tag="rdt")
                nc.sync.dma_start(sidx[:], srcidx[t])
                nc.sync.dma_start(didx[:], dstidx[t])
                nc.sync.dma_start(rdt[:], rdd[t])
                gs = gp.tile([BLK, NBLK, FEAT], bf16, tag="gs")
                gd = gp.tile([BLK, NBLK, FEAT], bf16, tag="gd")
                for b in range(NBLK):
                    nc.gpsimd.indirect_dma_start(
                        out=gs[:, b, :], out_offset=None, in_=nodesB[:],
                        in_offset=bass.IndirectOffsetOnAxis(ap=sidx[:, b:b + 1], axis=0))
                    nc.gpsimd.indirect_dma_start(
                        out=gd[:, b, :], out_offset=None, in_=nodesB[:],
                        in_offset=bass.IndirectOffsetOnAxis(ap=didx[:, b:b + 1], axis=0))

                # ============ edge-major tensor-product planes (DVE + Pool) ============
                # (emitted before the embedding chain: these feed the PE transposes
                # that open the tile, while sin/emb is only needed by df1 later)
                u = ep.tile([BLK, NBLK, NB], f32, tag="u")
                nc.vector.tensor_tensor(
                    out=u[:],
                    in0=rdt[:, :, 0:1].to_broadcast([BLK, NBLK, NB]),
                    in1=cn_t[:].unsqueeze(1).to_broadcast([BLK, NBLK, NB]),
                    op=OP.mult)
                e0m = pp.tile([BLK, NBLK, 1024], bf16, tag="e0m")
                e1m = pp.tile([BLK, NBLK, 256], bf16, tag="e1m")
                e2m = pp.tile([BLK, NBLK, 64], bf16, tag="e2m")
                for b in range(NBLK):
                    nc.vector.tensor_tensor(
                        out=e0m[:, b, :].rearrange("p (u v) -> p u v", v=L0),
                        in0=gs[:, b, 0:L0].unsqueeze(2).to_broadcast([BLK, L0, L0]),
                        in1=gd[:, b, 0:L0].unsqueeze(1).to_broadcast([BLK, L0, L0]),
                        op=OP.mult)
                    t1m = pp.tile([BLK, 256], bf16, tag="t1m")
                    for i in range(3):
                        s_ap = gs[:, b, O1:O1 + 3 * L1].rearrange(
                            "p (u i) -> p u i", i=3)[:, :, i:i + 1].to_broadcast([BLK, L1, L1])
                        d_ap = gd[:, b, O1:O1 + 3 * L1].rearrange(
                            "p (v i) -> p i v", i=3)[:, i:i + 1, :].to_broadcast([BLK, L1, L1])
                        dst = e1m[:, b, :] if i == 0 else t1m[:]
                        nc.vector.tensor_tensor(
                            out=dst.rearrange("p (u v) -> p u v", v=L1),
                            in0=s_ap, in1=d_ap, op=OP.mult)
                        if i > 0:
                            nc.vector.tensor_tensor(out=e1m[:, b, :], in0=e1m[:, b, :],
                                                    in1=t1m[:], op=OP.add)
                    t2m = pp.tile([BLK, 64], bf16, tag="t2m")
                    for i in range(5):
                        s_ap = gs[:, b, O2:O2 + 5 * L2].rearrange(
                            "p (u i) -> p u i", i=5)[:, :, i:i + 1].to_broadcast([BLK, L2, L2])
                        d_ap = gd[:, b, O2:O2 + 5 * L2].rearrange(
                            "p (v i) -> p i v", i=5)[:, i:i + 1, :].to_broadcast([BLK, L2, L2])
                        if i == 0:
                            nc.gpsimd.tensor_tensor(
                                out=e2m[:, b, :].rearrange("p (u v) -> p u v", v=L2),
                                in0=s_ap, in1=d_ap, op=OP.mult)
                        else:
                            nc.gpsimd.tensor_tensor(
                                out=t2m[:].rearrange("p (u v) -> p u v", v=L2),
                                in0=s_ap, in1=d_ap, op=OP.mult)
                            nc.gpsimd.tensor_tensor(out=e2m[:, b, :], in0=e2m[:, b, :],
                                                    in1=t2m[:], op=OP.add)

                # ============ embedding: sin(2*pi*frac(r*n/(2C))) * EMBC*dist ============
                icv = ep.tile([BLK, NBLK, NB], i32, tag="icv")
                nc.vector.tensor_copy(icv[:], u[:])
                nc.vector.tensor_tensor(out=u[:], in0=u[:], in1=icv[:], op=OP.subtract)
                sinv = eps_p.tile([BLK, NBLK, NB], bf16, tag="sinv")
                nc.scalar.activation(sinv[:], u[:], AF.Sin, bias=sinb_t[:, 0:1], scale=TWO_PI)
                for b in range(NBLK):
                    nc.vector.tensor_scalar(
                        out=sinv[:, b, :], in0=sinv[:, b, :],
                        scalar1=rdt[:, b, 1:2], scalar2=sim_neg,
                        op0=OP.mult, op1=OP.mult)
                hctx.__exit__(None, None, None)

                # ============ transposes to uv-major (PE, bf16) ============
                copy_flip = [0]

                def pscopy(dst, src):
                    # alternate PSUM->SBUF copies between Act and DVE
                    if copy_flip[0] % 2 == 0:
                        nc.scalar.copy(dst, src)
                    else:
                        nc.vector.tensor_copy(dst, src)
                    copy_flip[0] += 1

                trst = {"n": 0, "ptr": None}

                def trp(dst_ap, srcs):
                    # 2 pool bufs x 2 slots = 4 staging chunks in flight
                    if trst["n"] % 2 == 0:
                        trst["ptr"] = pst.tile([BLK, 2, ET], bf16, tag="tr", name="ptr")
                    s = trst["n"] % 2
                    trst["n"] += 1
                    ptr = trst["ptr"]
                    np_out = srcs[0].free_size()
                    for b, src in enumerate(srcs):
                        nc.tensor.transpose(ptr[:np_out, s,
                                                b * BLK:(b + 1) * BLK], src, id_t[:])
                    pscopy(dst_ap, ptr[:np_out, s, :])

                embT = xp.tile([BLK, 2, ET], bf16, tag="embT")
                for c in range(2):
                    trp(embT[:, c, :],
                        [sinv[:, b, c * BLK:(c + 1) * BLK] for b in range(NBLK)])
                e0T = xp.tile([BLK, 8, ET], bf16, tag="e0T")
                for c in range(8):
                    trp(e0T[:, c, :],
                        [e0m[:, b, c * BLK:(c + 1) * BLK] for b in range(NBLK)])
                e1T = xp.tile([BLK, 2, ET], bf16, tag="e1T")
                for c in range(2):
                    trp(e1T[:, c, :],
                        [e1m[:, b, c * BLK:(c + 1) * BLK] for b in range(NBLK)])
                e2T = xp.tile([64, ET], bf16, tag="e2T")
                trp(e2T[:], [e2m[:, b, :] for b in range(NBLK)])

                # ============ W-contraction -> mixed [512w x 512e] ============
                # m-outer with a 2-bank rotating accumulator: each 128-row w-chunk
                # of `mixed` finishes early and is immediately evacuated + squared;
                # the LN-stat matmuls are emitted after df1 so their inputs are
                # long since ready when PE reaches them (no PE stalls on copies).
                mix_sb = mp.tile([BLK, 4, NS], bf16, tag="mix_sb")
                sq_sb = mp.tile([BLK, 4, NS], bf16, tag="sq_sb")
                mu_ps = pso.tile([BLK, NS], f32, space="PSUM", tag="po", name="mu_ps")

                def stats_mm(m):
                    # LN mean matmul for chunk m, one chunk behind the
                    # contraction so the evacuation copies are already done.
                    nc.tensor.matmul(mu_ps[:], ones_t[:], mix_sb[:, m, :],
                                     start=(m == 0), stop=(m == 3),
                                     skip_group_check=True)

                for m in range(4):
                    mix_ps = psm.tile([BLK, NS], f32, space="PSUM", tag="mix")
                    for c in range(8):
                        nc.tensor.matmul(mix_ps[:],
                                         w0_t[:, c, m * BLK:(m + 1) * BLK],
                                         e0T[:, c, :], start=(c == 0), stop=False)
                    for c in range(2):
                        nc.tensor.matmul(mix_ps[:],
                                         w1_t[:, c, m * BLK:(m + 1) * BLK],
                                         e1T[:, c, :], start=False, stop=False)
                    nc.tensor.matmul(mix_ps[:],
                                     w2_t[:, m * BLK:(m + 1) * BLK],
                                     e2T[:], start=False, stop=True)
                    pscopy(mix_sb[:, m, :], mix_ps[:])
                    nc.vector.tensor_tensor(out=sq_sb[:, m, :], in0=mix_sb[:, m, :],
                                            in1=mix_sb[:, m, :], op=OP.mult)
                    if m >= 1:
                        stats_mm(m - 1)
                stats_mm(3)
                mu_n = sp.tile([BLK, NS], f32, tag="mu_n")
                nc.vector.tensor_scalar(out=mu_n[:], in0=mu_ps[:], scalar1=1.0 / NS,
                                        scalar2=None, op0=OP.mult)
                s2_ps = pso.tile([BLK, NS], f32, space="PSUM", tag="po", name="s2_ps")
                for m in range(4):
                    nc.tensor.matmul(s2_ps[:], ones_t[:], sq_sb[:, m, :],
                                     start=(m == 0), stop=(m == 3))
                s2_n = sp.tile([BLK, NS], f32, tag="s2_n")
                nc.vector.tensor_scalar(out=s2_n[:], in0=s2_ps[:], scalar1=1.0 / NS,
                                        scalar2=1e-5, op0=OP.mult, op1=OP.add)

                # ============ distance-filter MLP, first layer ============
                h1c = hp.tile([BLK, 8, ET], bf16, tag="h1c")
                for m in range(8):
                    ph = psh.tile([BLK, NS], f32, space="PSUM", tag="ph")
                    for c in range(2):
                        nc.tensor.matmul(ph[:], dfw1_t[:, c, m * BLK:(m + 1) * BLK],
                                         embT[:, c, :], start=(c == 0), stop=(c == 1))
                    silu_to(h1c[:, m, :], ph[:], bdf1_t[:, m:m + 1])
                var = sp.tile([BLK, NS], f32, tag="var")
                nc.vector.tensor_tensor(out=var[:], in0=mu_n[:], in1=mu_n[:], op=OP.mult)
                nc.vector.tensor_tensor(out=var[:], in0=s2_n[:], in1=var[:], op=OP.subtract)
                # Newton rsqrt (magic-seed) on DVE: keeps the tile loop inside one
                # act-function table (silu_and_others) -- no per-tile table reloads
                yf = sp.tile([BLK, NS], f32, tag="yf")
                nc.vector.tensor_scalar(out=yf[:].bitcast(i32), in0=var[:].bitcast(i32),
                                        scalar1=1, scalar2=None, op0=OP.arith_shift_right)
                nc.vector.tensor_scalar(out=yf[:].bitcast(i32), in0=yf[:].bitcast(i32),
                                        scalar1=0x5F3759DF, scalar2=-1,
                                        op0=OP.subtract, op1=OP.mult)
                tb = sp.tile([BLK, NS], f32, tag="tb")
                yb = sp.tile([BLK, NS], bf16, tag="yb")
                for it in range(2):
                    nc.vector.tensor_tensor(out=tb[:], in0=yf[:], in1=yf[:], op=OP.mult)
                    nc.vector.tensor_tensor(out=tb[:], in0=tb[:], in1=var[:], op=OP.mult)
                    nc.vector.tensor_scalar(out=tb[:], in0=tb[:], scalar1=-0.5, scalar2=1.5,
                                            op0=OP.mult, op1=OP.add)
                    dst = yf[:] if it == 0 else yb[:]
                    nc.vector.tensor_tensor(out=dst, in0=yf[:], in1=tb[:], op=OP.mult)
                mub = sp.tile([BLK, NS], bf16, tag="mub")
                nc.vector.tensor_tensor(out=mub[:], in0=mu_n[:], in1=yb[:], op=OP.mult)

                # ============ distance-filter MLP, second layer ============
                dff = mp.tile([BLK, 4, NS], bf16, tag="dff")
                for m in range(4):
                    df_ps = psm.tile([BLK, NS], f32, space="PSUM", tag="mix")
                    for kc in range(8):
                        nc.tensor.matmul(df_ps[:],
                                         dfw2_t[:, kc, m * BLK:(m + 1) * BLK],
                                         h1c[:, kc, :], start=(kc == 0), stop=(kc == 7))
                    if m % 2 == 0:
                        nc.scalar.activation(dff[:, m, :], df_ps[:], AF.Identity,
                                             bias=bdf2_t[:, m:m + 1], scale=1.0)
                    else:
                        nc.vector.tensor_scalar(out=dff[:, m, :], in0=df_ps[:],
                                                scalar1=bdf2_t[:, m:m + 1],
                                                scalar2=None, op0=OP.add)

                # ============ reg = (mix - mu)*rstd*df  (as mix*yb - mub, then *df) ==
                reg = mp.tile([BLK, 4, NS], bf16, tag="reg")
                for m in range(4):
                    tr1 = sp.tile([BLK, NS], bf16, tag="tr1")
                    nc.vector.tensor_tensor(out=tr1[:], in0=mix_sb[:, m, :], in1=yb[:],
                                            op=OP.mult)
                    tr2 = sp.tile([BLK, NS], bf16, tag="tr2")
                    nc.vector.tensor_tensor(out=tr2[:], in0=tr1[:], in1=mub[:],
                                            op=OP.subtract)
                    nc.vector.tensor_tensor(out=reg[:, m, :], in0=tr2[:], in1=dff[:, m, :],
                                            op=OP.mult)

                # ============ mix MLP ============
                h = hp.tile([BLK, 8, ET], bf16, tag="h")
                for m in range(8):
                    ph = psh.tile([BLK, NS], f32, space="PSUM", tag="ph")
                    for kc in range(4):
                        nc.tensor.matmul(ph[:], miw1_t[:, kc, m * BLK:(m + 1) * BLK],
                                         reg[:, kc, :], start=(kc == 0), stop=(kc == 3))
                    silu_to(h[:, m, :], ph[:], bmi1_t[:, m:m + 1])
                # mow contraction: accumulate h2m(m)*mow[:,m] on DVE (one
                # scalar_tensor_tensor per m), then a single cross-partition
                # ones-matmul -- saves 7 PE matmuls per tile
                po = pso.tile([BLK, NS], f32, space="PSUM", tag="po")
                oacc = sp.tile([BLK, ET], f32, tag="oacc")
                for m in range(8):
                    ph = psh.tile([BLK, NS], f32, space="PSUM", tag="ph")
                    for kc in range(8):
                        nc.tensor.matmul(ph[:], miw2_t[:, kc, m * BLK:(m + 1) * BLK],
                                         h[:, kc, :], start=(kc == 0), stop=(kc == 7))
                    h2m = sp.tile([BLK, ET], bf16, tag="h2m")
                    silu_to(h2m[:], ph[:], bmi2_t[:, m:m + 1])
                    if m == 0:
                        nc.vector.tensor_scalar(out=oacc[:], in0=h2m[:],
                                                scalar1=mowf_t[:, 0:1], scalar2=None,
                                                op0=OP.mult)
                    else:
                        nc.vector.scalar_tensor_tensor(
                            out=oacc[:], in0=h2m[:], scalar=mowf_t[:, m:m + 1],
                            in1=oacc[:], op0=OP.mult, op1=OP.add)
                nc.tensor.matmul(po[0:1, :], ones32_t[:], oacc[:],
                                 start=True, stop=True)
                ot = sp.tile([1, ET], f32, tag="ot")
                nc.scalar.activation(ot[:], po[0:1, :], AF.Identity,
                                     bias=bmo_t[:, 0:1], scale=1.0)
                nc.sync.dma_start(outd[t], ot[:])

    nc.finalize()
    return nc


def _np_bf16():
    import concourse.mybir as mybir
    return mybir.dt.np(mybir.dt.bfloat16)


def _host_prep(inputs):
    """Shared (replicated) host-side tensors."""
    f = np.float32
    bf = _np_bf16()
    nodes = np.asarray(inputs["nodes"], f)
    W0 = np.asarray(inputs["W0"], f)
    W1 = np.asarray(inputs["W1"], f)
    W2 = np.asarray(inputs["W2"], f)
    ln_g = np.asarray(inputs["ln_g"], f)

    sym = lambda W: 0.5 * (W + W.transpose(1, 0, 2))
    w0f = np.ascontiguousarray((sym(W0) / FAN).reshape(L0 * L0, NS))
    w1f = np.ascontiguousarray((sym(W1) / (FAN * math.sqrt(3.0))).reshape(L1 * L1, NS))
    w2f = np.ascontiguousarray((sym(W2) / (FAN * math.sqrt(5.0))).reshape(L2 * L2, NS))
    miw1 = np.ascontiguousarray(ln_g[:, None] * np.asarray(inputs["mi_w1"], f))

    def colbias(b, nch):
        b = np.asarray(b, f).reshape(nch, BLK)
        return np.ascontiguousarray(b.T)

    cn = np.broadcast_to((np.arange(1, NB + 1, dtype=f) / (2.0 * CUT))[None, :],
                         (BLK, NB)).copy()

    return dict(
        nodesB=nodes.astype(bf),
        w0d=w0f.astype(bf), w1d=w1f.astype(bf), w2d=w2f.astype(bf),
        dfw1d=np.asarray(inputs["df_w1"], f).astype(bf),
        dfw2d=np.asarray(inputs["df_w2"], f).astype(bf),
        miw1d=miw1.astype(bf),
        miw2d=np.asarray(inputs["mi_w2"], f).astype(bf),
        mowd=np.asarray(inputs["mo_w"], f),
        bdf1=colbias(inputs["df_b1"], 8), bdf2=colbias(inputs["df_b2"], 4),
        bmi1=colbias(inputs["mi_b1"], 8), bmi2=colbias(inputs["mi_b2"], 8),
        bmo=np.asarray(inputs["mo_b"], f).reshape(1, 1),
        cnd=cn,
        identd=np.eye(BLK, dtype=f).astype(bf),
        onesd=np.ones((BLK, BLK), f).astype(bf),
    )


def _edge_prep(inputs, core, ntiles):
    """Per-core edge tensors: tiled indices + per-edge radial scalars."""
    f = np.float32
    ec = ntiles * ET
    lo = core * EC
    ei = np.asarray(inputs["edge_index"])
    src = ei[0, lo:lo + ec].astype(np.int32)
    dst = ei[1, lo:lo + ec].astype(np.int32)
    bv = np.asarray(inputs["batch_vec"]).astype(np.int32)
    shift = np.asarray(inputs["edge_shift"], f)[lo:lo + ec]
    pos = np.asarray(inputs["pos"], f)
    cell = np.asarray(inputs["cell"], f)

    bcell = cell[bv[src]]                              # (ec,3,3)
    tvec = np.einsum('ei,eij->ej', shift, bcell)
    radvec = pos[dst] - pos[src] + tvec
    dist = np.sqrt((radvec * radvec).sum(1)) + 1e-6    # (ec,)
    r = 1.0 / dist
    rd = np.stack([r, EMBC * dist], axis=1).astype(f)  # (ec, 2)

    def tile_idx(x):
        return np.ascontiguousarray(x.reshape(ntiles, NBLK, BLK).transpose(0, 2, 1))

    return dict(
        srcidx=tile_idx(src), dstidx=tile_idx(dst),
        rdd=np.ascontiguousarray(
            rd.reshape(ntiles, NBLK, BLK, 2).transpose(0, 2, 1, 3)),
    )


def _run(inputs, mode, ntiles, ncores):
    key = (mode, ntiles, 1)
    if key not in _cache:
        _cache[key] = _build(mode, ntiles)
    nc = _cache[key]
    shared = _host_prep(inputs)
    in_maps = []
    for c in range(ncores):
        m = dict(shared)
        m.update(_edge_prep(inputs, c, ntiles))
        in_maps.append(m)

    if mode == "sim":
        from concourse.bass_interp import CoreSim
        outs = []
        for c in range(ncores):
            sim = CoreSim(nc)
            for k, v in in_maps[c].items():
                sim.tensor(k)[:] = v
            sim.simulate()
            outs.append(np.array(sim.tensor("out")).reshape(-1))
        return np.concatenate(outs).reshape(-1, 1)

    from concourse.bass_utils import run_bass_kernel_spmd
    trace = os.environ.get("EXB_TRACE", "0") == "1"
    res = run_bass_kernel_spmd(nc, in_maps, list(range(ncores)), trace=trace)
    out = np.concatenate([res.results[c]["out"].reshape(-1) for c in range(ncores)])
    if trace:
        _run.last_exec_time_ns = res.exec_time_ns
    return out.reshape(-1, 1)


def kernel(**inputs) -> np.ndarray:
    return _run(inputs, os.environ.get("EXB_MODE", "hw"), EC // ET, NCORES).astype(np.float32)


# revision 4
# speedup vs baseline: 1.4951x; 1.4951x over previous
"""Trainium2 Bass kernel for nn_ExchangeBlock (gnn_message_passing), v2.

Data-parallel over edges: each of the 8 cores processes E/8 = 16384 edges in
32 tiles of 512. Per tile:
  - indirect-DMA gather of bf16 node rows for src/dst (radial scalars r and
    EMBC*dist are precomputed per edge on the host and shipped directly);
  - Bessel embedding via range-reduced Sin on the Act engine (the whole tile
    loop stays inside the one `silu_and_others` act table: no table reloads);
  - tensor product computed EDGE-MAJOR: the outer products e0/e1/e2 are pure
    broadcast-AP elementwise ops on DVE (bf16, 2x mode), then PE-transposed
    (bf16, 1 cycle/row) to uv-major for the W-contraction, m-outer with per-m
    evacuation so LN stats overlap the df MLP;
  - LayerNorm: all-ones 128x128 stat matmuls (broadcast built in) + DVE
    Newton-rsqrt;
  - df/mix MLPs as bf16 matmuls with all weights SBUF-resident; the final
    mo_w contraction runs as DVE scalar_tensor_tensor + one ones-matmul.
Per-engine budget (cost model, per tile): PE ~44us (90% busy), DVE ~22us,
Act ~21us; next-tile gather/product front-end is priority-hoisted over the
current tile's MLP tail.
"""
import os
import sys

sys.path.insert(0, "/opt/trn_rl_repo")

import math
import numpy as np

L0, L1, L2 = 32, 16, 8
NS, NB = 512, 256
CUT = 7.0
N, E, G = 16384, 131072, 16
FEAT = L0 + 3 * L1 + 5 * L2  # 120
NCORES = 8
EC = E // NCORES
BLK = 128
ET = 512
NBLK = ET // BLK
FAN = math.sqrt(float(L0 * L0 + L1 * L1 + L2 * L2))
EMBC = math.sqrt(2.0 / CUT)
O1 = L0
O2 = L0 + 3 * L1

_cache = {}


def _build(mode, ntiles, reps=1):
    import concourse.bacc as bacc
    import concourse.bass as bass
    import concourse.mybir as mybir
    import concourse.tile as tile

    f32 = mybir.dt.float32
    bf16 = mybir.dt.bfloat16
    i32 = mybir.dt.int32
    AF = mybir.ActivationFunctionType
    OP = mybir.AluOpType

    nc = bacc.Bacc(None)

    # ---------------- DRAM tensors ----------------
    nodesB = nc.dram_tensor("nodesB", [N, FEAT], bf16, kind="ExternalInput")
    srcidx = nc.dram_tensor("srcidx", [ntiles, BLK, NBLK], i32, kind="ExternalInput")
    dstidx = nc.dram_tensor("dstidx", [ntiles, BLK, NBLK], i32, kind="ExternalInput")
    rdd = nc.dram_tensor("rdd", [ntiles, BLK, NBLK, 2], f32, kind="ExternalInput")

    w0d = nc.dram_tensor("w0d", [1024, NS], bf16, kind="ExternalInput")
    w1d = nc.dram_tensor("w1d", [256, NS], bf16, kind="ExternalInput")
    w2d = nc.dram_tensor("w2d", [64, NS], bf16, kind="ExternalInput")
    dfw1d = nc.dram_tensor("dfw1d", [256, 1024], bf16, kind="ExternalInput")
    dfw2d = nc.dram_tensor("dfw2d", [1024, NS], bf16, kind="ExternalInput")
    miw1d = nc.dram_tensor("miw1d", [NS, 1024], bf16, kind="ExternalInput")
    miw2d = nc.dram_tensor("miw2d", [1024, 1024], bf16, kind="ExternalInput")
    mowd = nc.dram_tensor("mowd", [1024, 1], f32, kind="ExternalInput")
    bdf1 = nc.dram_tensor("bdf1", [BLK, 8], f32, kind="ExternalInput")
    bdf2 = nc.dram_tensor("bdf2", [BLK, 4], f32, kind="ExternalInput")
    bmi1 = nc.dram_tensor("bmi1", [BLK, 8], f32, kind="ExternalInput")
    bmi2 = nc.dram_tensor("bmi2", [BLK, 8], f32, kind="ExternalInput")
    bmo = nc.dram_tensor("bmo", [1, 1], f32, kind="ExternalInput")
    cnd = nc.dram_tensor("cnd", [BLK, NB], f32, kind="ExternalInput")
    identd = nc.dram_tensor("identd", [BLK, BLK], bf16, kind="ExternalInput")
    onesd = nc.dram_tensor("onesd", [BLK, BLK], bf16, kind="ExternalInput")

    outd = nc.dram_tensor("out", [ntiles, 1, ET], f32, kind="ExternalOutput")

    TWO_PI = 2.0 * math.pi
    sin_bias = -math.pi if mode == "sim" else 0.0
    sim_neg = -1.0 if mode == "sim" else 1.0

    with tile.TileContext(nc) as tc:
        with (
            tc.tile_pool(name="const", bufs=1) as cp,
            tc.tile_pool(name="gat", bufs=2) as gp,
            tc.tile_pool(name="emb", bufs=1) as ep,
            tc.tile_pool(name="embs", bufs=2) as eps_p,
            tc.tile_pool(name="eprod", bufs=2) as pp,
            tc.tile_pool(name="etr", bufs=1) as xp,
            tc.tile_pool(name="mid", bufs=1) as mp,
            tc.tile_pool(name="sml", bufs=2) as sp,
            tc.tile_pool(name="act", bufs=1) as hp,
            tc.tile_pool(name="pstr", bufs=2, space="PSUM") as pst,
            tc.tile_pool(name="psph", bufs=3, space="PSUM") as psh,
            tc.tile_pool(name="pspo", bufs=1, space="PSUM") as pso,
            tc.tile_pool(name="psmix", bufs=2, space="PSUM") as psm,
        ):
            # ---------------- constants ----------------
            w0_t = cp.tile([BLK, 8, NS], bf16)
            for c in range(8):
                nc.sync.dma_start(w0_t[:, c, :], w0d[c * BLK:(c + 1) * BLK, :])
            w1_t = cp.tile([BLK, 2, NS], bf16)
            for c in range(2):
                nc.sync.dma_start(w1_t[:, c, :], w1d[c * BLK:(c + 1) * BLK, :])
            w2_t = cp.tile([64, NS], bf16)
            nc.sync.dma_start(w2_t[:], w2d[:])
            dfw1_t = cp.tile([BLK, 2, 1024], bf16)
            for c in range(2):
                nc.sync.dma_start(dfw1_t[:, c, :], dfw1d[c * BLK:(c + 1) * BLK, :])
            dfw2_t = cp.tile([BLK, 8, NS], bf16)
            for c in range(8):
                nc.sync.dma_start(dfw2_t[:, c, :], dfw2d[c * BLK:(c + 1) * BLK, :])
            miw1_t = cp.tile([BLK, 4, 1024], bf16)
            for c in range(4):
                nc.sync.dma_start(miw1_t[:, c, :], miw1d[c * BLK:(c + 1) * BLK, :])
            miw2_t = cp.tile([BLK, 8, 1024], bf16)
            for c in range(8):
                nc.sync.dma_start(miw2_t[:, c, :], miw2d[c * BLK:(c + 1) * BLK, :])
            mowf_t = cp.tile([BLK, 8], f32)
            nc.sync.dma_start(mowf_t[:], mowd[:].rearrange("(c p) one -> p (c one)", p=BLK))
            ones32_t = cp.tile([BLK, 1], f32)
            nc.gpsimd.memset(ones32_t[:], 1.0)
            bdf1_t = cp.tile([BLK, 8], f32)
            nc.sync.dma_start(bdf1_t[:], bdf1[:])
            bdf2_t = cp.tile([BLK, 4], f32)
            nc.sync.dma_start(bdf2_t[:], bdf2[:])
            bmi1_t = cp.tile([BLK, 8], f32)
            nc.sync.dma_start(bmi1_t[:], bmi1[:])
            bmi2_t = cp.tile([BLK, 8], f32)
            nc.sync.dma_start(bmi2_t[:], bmi2[:])
            bmo_t = cp.tile([1, 1], f32)
            nc.sync.dma_start(bmo_t[:], bmo[:])
            cn_t = cp.tile([BLK, NB], f32)
            nc.sync.dma_start(cn_t[:], cnd[:])
            id_t = cp.tile([BLK, BLK], bf16)
            nc.sync.dma_start(id_t[:], identd[:])
            ones_t = cp.tile([BLK, BLK], bf16)
            nc.sync.dma_start(ones_t[:], onesd[:])
            sinb_t = cp.tile([BLK, 1], f32)
            nc.gpsimd.memset(sinb_t[:], sin_bias)
            eps_t = cp.tile([BLK, 1], f32)
            nc.gpsimd.memset(eps_t[:], 1e-5)

            def silu_to(dst, ps, bias_ap):
                if mode == "sim":
                    sg = sp.tile([BLK, ET], f32, tag="sg")
                    nc.scalar.activation(sg[:], ps, AF.Sigmoid, bias=bias_ap, scale=1.0)
                    pre = sp.tile([BLK, ET], f32, tag="pre")
                    nc.vector.tensor_scalar(out=pre[:], in0=ps, scalar1=bias_ap,
                                            scalar2=None, op0=OP.add)
                    nc.vector.tensor_tensor(out=dst, in0=sg[:], in1=pre[:], op=OP.mult)
                else:
                    nc.scalar.activation(dst, ps, AF.Silu, bias=bias_ap, scale=1.0)

            for t in [tt for _ in range(reps) for tt in range(ntiles)]:
                # ============ index / scalar loads + gathers ============
                # high_priority: let tile t's gather/emb/TP-product front-end
                # schedule ahead of tile t-1's MLP tail so DMA/DVE/Pool overlap PE
                hctx = tc.high_priority(100)
                hctx.__enter__()
                sidx = gp.tile([BLK, NBLK], i32, tag="sidx")
                didx = gp.tile([BLK, NBLK], i32, tag="didx")
                rdt = gp.tile([BLK, NBLK, 2], f32, tag="rdt")
                nc.sync.dma_start(sidx[:], srcidx[t])
                nc.sync.dma_start(didx[:], dstidx[t])
                nc.sync.dma_start(rdt[:], rdd[t])
                gs = gp.tile([BLK, NBLK, FEAT], bf16, tag="gs")
                gd = gp.tile([BLK, NBLK, FEAT], bf16, tag="gd")
                for b in range(NBLK):
                    nc.gpsimd.indirect_dma_start(
                        out=gs[:, b, :], out_offset=None, in_=nodesB[:],
                        in_offset=bass.IndirectOffsetOnAxis(ap=sidx[:, b:b + 1], axis=0))
                    nc.gpsimd.indirect_dma_start(
                        out=gd[:, b, :], out_offset=None, in_=nodesB[:],
                        in_offset=bass.IndirectOffsetOnAxis(ap=didx[:, b:b + 1], axis=0))

                # ============ edge-major tensor-product planes (DVE + Pool) ============
                # (emitted before the embedding chain: these feed the PE transposes
                # that open the tile, while sin/emb is only needed by df1 later)
                u = ep.tile([BLK, NBLK, NB], f32, tag="u")
                nc.vector.tensor_tensor(
                    out=u[:],
                    in0=rdt[:, :, 0:1].to_broadcast([BLK, NBLK, NB]),
                    in1=cn_t[:].unsqueeze(1).to_broadcast([BLK, NBLK, NB]),
                    op=OP.mult)
                e0m = pp.tile([BLK, NBLK, 1024], bf16, tag="e0m")
                e1m = pp.tile([BLK, NBLK, 256], bf16, tag="e1m")
                e2m = pp.tile([BLK, NBLK, 64], bf16, tag="e2m")
                for b in range(NBLK):
                    nc.vector.tensor_tensor(
                        out=e0m[:, b, :].rearrange("p (u v) -> p u v", v=L0),
                        in0=gs[:, b, 0:L0].unsqueeze(2).to_broadcast([BLK, L0, L0]),
                        in1=gd[:, b, 0:L0].unsqueeze(1).to_broadcast([BLK, L0, L0]),
                        op=OP.mult)
                    t1m = pp.tile([BLK, 256], bf16, tag="t1m")
                    for i in range(3):
                        s_ap = gs[:, b, O1:O1 + 3 * L1].rearrange(
                            "p (u i) -> p u i", i=3)[:, :, i:i + 1].to_broadcast([BLK, L1, L1])
                        d_ap = gd[:, b, O1:O1 + 3 * L1].rearrange(
                            "p (v i) -> p i v", i=3)[:, i:i + 1, :].to_broadcast([BLK, L1, L1])
                        dst = e1m[:, b, :] if i == 0 else t1m[:]
                        nc.vector.tensor_tensor(
                            out=dst.rearrange("p (u v) -> p u v", v=L1),
                            in0=s_ap, in1=d_ap, op=OP.mult)
                        if i > 0:
                            nc.vector.tensor_tensor(out=e1m[:, b, :], in0=e1m[:, b, :],
                                                    in1=t1m[:], op=OP.add)
                    t2m = pp.tile([BLK, 64], bf16, tag="t2m")
                    for i in range(5):
                        s_ap = gs[:, b, O2:O2 + 5 * L2].rearrange(
                            "p (u i) -> p u i", i=5)[:, :, i:i + 1].to_broadcast([BLK, L2, L2])
                        d_ap = gd[:, b, O2:O2 + 5 * L2].rearrange(
                            "p (v i) -> p i v", i=5)[:, i:i + 1, :].to_broadcast([BLK, L2, L2])
                        if i == 0:
                            nc.gpsimd.tensor_tensor(
                                out=e2m[:, b, :].rearrange("p (u v) -> p u v", v=L2),
                                in0=s_ap, in1=d_ap, op=OP.mult)
                        else:
                            nc.gpsimd.tensor_tensor(
                                out=t2m[:].rearrange("p (u v) -> p u v", v=L2),
                                in0=s_ap, in1=d_ap, op=OP.mult)
                            nc.gpsimd.tensor_tensor(out=e2m[:, b, :], in0=e2m[:, b, :],
                                                    in1=t2m[:], op=OP.add)

                # ============ embedding: sin(2*pi*frac(r*n/(2C))) * EMBC*dist ============
                icv = ep.tile([BLK, NBLK, NB], i32, tag="icv")
                nc.vector.tensor_copy(icv[:], u[:])
                nc.vector.tensor_tensor(out=u[:], in0=u[:], in1=icv[:], op=OP.subtract)
                sinv = eps_p.tile([BLK, NBLK, NB], bf16, tag="sinv")
                nc.scalar.activation(sinv[:], u[:], AF.Sin, bias=sinb_t[:, 0:1], scale=TWO_PI)
                for b in range(NBLK):
                    nc.vector.tensor_scalar(
                        out=sinv[:, b, :], in0=sinv[:, b, :],
                        scalar1=rdt[:, b, 1:2], scalar2=sim_neg,
                        op0=OP.mult, op1=OP.mult)
                hctx.__exit__(None, None, None)

                # ============ transposes to uv-major (PE, bf16) ============
                copy_flip = [0]

                def pscopy(dst, src):
                    # alternate PSUM->SBUF copies between Act and DVE
                    if copy_flip[0] % 2 == 0:
                        nc.scalar.copy(dst, src)
                    else:
                        nc.vector.tensor_copy(dst, src)
                    copy_flip[0] += 1

                trst = {"n": 0, "ptr": None}

                def trp(dst_ap, srcs):
                    # 2 pool bufs x 2 slots = 4 staging chunks in flight
                    if trst["n"] % 2 == 0:
                        trst["ptr"] = pst.tile([BLK, 2, ET], bf16, tag="tr", name="ptr")
                    s = trst["n"] % 2
                    trst["n"] += 1
                    ptr = trst["ptr"]
                    np_out = srcs[0].free_size()
                    for b, src in enumerate(srcs):
                        nc.tensor.transpose(ptr[:np_out, s,
                                                b * BLK:(b + 1) * BLK], src, id_t[:])
                    pscopy(dst_ap, ptr[:np_out, s, :])

                embT = xp.tile([BLK, 2, ET], bf16, tag="embT")
                for c in range(2):
                    trp(embT[:, c, :],
                        [sinv[:, b, c * BLK:(c + 1) * BLK] for b in range(NBLK)])
                e0T = xp.tile([BLK, 8, ET], bf16, tag="e0T")
                for c in range(8):
                    trp(e0T[:, c, :],
                        [e0m[:, b, c * BLK:(c + 1) * BLK] for b in range(NBLK)])
                e1T = xp.tile([BLK, 2, ET], bf16, tag="e1T")
                for c in range(2):
                    trp(e1T[:, c, :],
                        [e1m[:, b, c * BLK:(c + 1) * BLK] for b in range(NBLK)])
                e2T = xp.tile([64, ET], bf16, tag="e2T")
                trp(e2T[:], [e2m[:, b, :] for b in range(NBLK)])

                # ============ W-contraction -> mixed [512w x 512e] ============
                # m-outer with a 2-bank rotating accumulator: each 128-row w-chunk
                # of `mixed` finishes early and is immediately evacuated + squared;
                # the LN-stat matmuls are emitted after df1 so their inputs are
                # long since ready when PE reaches them (no PE stalls on copies).
                mix_sb = mp.tile([BLK, 4, NS], bf16, tag="mix_sb")
                sq_sb = mp.tile([BLK, 4, NS], bf16, tag="sq_sb")
                mu_ps = pso.tile([BLK, NS], f32, space="PSUM", tag="po", name="mu_ps")

                def stats_mm(m):
                    # LN mean matmul for chunk m, one chunk behind the
                    # contraction so the evacuation copies are already done.
                    nc.tensor.matmul(mu_ps[:], ones_t[:], mix_sb[:, m, :],
                                     start=(m == 0), stop=(m == 3),
                                     skip_group_check=True)

                for m in range(4):
                    mix_ps = psm.tile([BLK, NS], f32, space="PSUM", tag="mix")
                    for c in range(8):
                        nc.tensor.matmul(mix_ps[:],
                                         w0_t[:, c, m * BLK:(m + 1) * BLK],
                                         e0T[:, c, :], start=(c == 0), stop=False)
                    for c in range(2):
                        nc.tensor.matmul(mix_ps[:],
                                         w1_t[:, c, m * BLK:(m + 1) * BLK],
                                         e1T[:, c, :], start=False, stop=False)
                    nc.tensor.matmul(mix_ps[:],
                                     w2_t[:, m * BLK:(m + 1) * BLK],
                                     e2T[:], start=False, stop=True)
                    pscopy(mix_sb[:, m, :], mix_ps[:])
                    nc.vector.tensor_tensor(out=sq_sb[:, m, :], in0=mix_sb[:, m, :],
                                            in1=mix_sb[:, m, :], op=OP.mult)
                    if m >= 1:
                        stats_mm(m - 1)
                stats_mm(3)
                mu_n = sp.tile([BLK, NS], f32, tag="mu_n")
                nc.vector.tensor_scalar(out=mu_n[:], in0=mu_ps[:], scalar1=1.0 / NS,
                                        scalar2=None, op0=OP.mult)
                s2_ps = pso.tile([BLK, NS], f32, space="PSUM", tag="po", name="s2_ps")
                for m in range(4):
                    nc.tensor.matmul(s2_ps[:], ones_t[:], sq_sb[:, m, :],
                                     start=(m == 0), stop=(m == 3))
                s2_n = sp.tile([BLK, NS], f32, tag="s2_n")
                nc.vector.tensor_scalar(out=s2_n[:], in0=s2_ps[:], scalar1=1.0 / NS,
                                        scalar2=1e-5, op0=OP.mult, op1=OP.add)

                # ============ distance-filter MLP, first layer ============
                h1c = hp.tile([BLK, 8, ET], bf16, tag="h1c")
                for m in range(8):
                    ph = psh.tile([BLK, NS], f32, space="PSUM", tag="ph")
                    for c in range(2):
                        nc.tensor.matmul(ph[:], dfw1_t[:, c, m * BLK:(m + 1) * BLK],
                                         embT[:, c, :], start=(c == 0), stop=(c == 1))
                    silu_to(h1c[:, m, :], ph[:], bdf1_t[:, m:m + 1])
                var = sp.tile([BLK, NS], f32, tag="var")
                nc.vector.tensor_tensor(out=var[:], in0=mu_n[:], in1=mu_n[:], op=OP.mult)
                nc.vector.tensor_tensor(out=var[:], in0=s2_n[:], in1=var[:], op=OP.subtract)
                # Newton rsqrt (magic-seed) on DVE: keeps the tile loop inside one
                # act-function table (silu_and_others) -- no per-tile table reloads
                yf = sp.tile([BLK, NS], f32, tag="yf")
                nc.vector.tensor_scalar(out=yf[:].bitcast(i32), in0=var[:].bitcast(i32),
                                        scalar1=1, scalar2=None, op0=OP.arith_shift_right)
                nc.vector.tensor_scalar(out=yf[:].bitcast(i32), in0=yf[:].bitcast(i32),
                                        scalar1=0x5F3759DF, scalar2=-1,
                                        op0=OP.subtract, op1=OP.mult)
                tb = sp.tile([BLK, NS], f32, tag="tb")
                yb = sp.tile([BLK, NS], bf16, tag="yb")
                for it in range(2):
                    nc.vector.tensor_tensor(out=tb[:], in0=yf[:], in1=yf[:], op=OP.mult)
                    nc.vector.tensor_tensor(out=tb[:], in0=tb[:], in1=var[:], op=OP.mult)
                    nc.vector.tensor_scalar(out=tb[:], in0=tb[:], scalar1=-0.5, scalar2=1.5,
                                            op0=OP.mult, op1=OP.add)
                    dst = yf[:] if it == 0 else yb[:]
                    nc.vector.tensor_tensor(out=dst, in0=yf[:], in1=tb[:], op=OP.mult)
                mub = sp.tile([BLK, NS], bf16, tag="mub")
                nc.vector.tensor_tensor(out=mub[:], in0=mu_n[:], in1=yb[:], op=OP.mult)

                # ============ distance-filter MLP, second layer ============
                dff = mp.tile([BLK, 4, NS], bf16, tag="dff")
                for m in range(4):
                    df_ps = psm.tile([BLK, NS], f32, space="PSUM", tag="mix")
                    for kc in range(8):
                        nc.tensor.matmul(df_ps[:],
                                         dfw2_t[:, kc, m * BLK:(m + 1) * BLK],
                                         h1c[:, kc, :], start=(kc == 0), stop=(kc == 7))
                    if m % 2 == 0:
                        nc.scalar.activation(dff[:, m, :], df_ps[:], AF.Identity,
                                             bias=bdf2_t[:, m:m + 1], scale=1.0)
                    else:
                        nc.vector.tensor_scalar(out=dff[:, m, :], in0=df_ps[:],
                                                scalar1=bdf2_t[:, m:m + 1],
                                                scalar2=None, op0=OP.add)

                # ============ reg = (mix - mu)*rstd*df  (as mix*yb - mub, then *df) ==
                reg = mp.tile([BLK, 4, NS], bf16, tag="reg")
                for m in range(4):
                    tr1 = sp.tile([BLK, NS], bf16, tag="tr1")
                    nc.vector.tensor_tensor(out=tr1[:], in0=mix_sb[:, m, :], in1=yb[:],
                                            op=OP.mult)
                    tr2 = sp.tile([BLK, NS], bf16, tag="tr2")
                    nc.vector.tensor_tensor(out=tr2[:], in0=tr1[:], in1=mub[:],
                                            op=OP.subtract)
                    nc.vector.tensor_tensor(out=reg[:, m, :], in0=tr2[:], in1=dff[:, m, :],
                                            op=OP.mult)

                # ============ mix MLP ============
                h = hp.tile([BLK, 8, ET], bf16, tag="h")
                for m in range(8):
                    ph = psh.tile([BLK, NS], f32, space="PSUM", tag="ph")
                    for kc in range(4):
                        nc.tensor.matmul(ph[:], miw1_t[:, kc, m * BLK:(m + 1) * BLK],
                                         reg[:, kc, :], start=(kc == 0), stop=(kc == 3))
                    silu_to(h[:, m, :], ph[:], bmi1_t[:, m:m + 1])
                # mow contraction: accumulate h2m(m)*mow[:,m] on DVE (one
                # scalar_tensor_tensor per m), then a single cross-partition
                # ones-matmul -- saves 7 PE matmuls per tile
                po = pso.tile([BLK, NS], f32, space="PSUM", tag="po")
                oacc = sp.tile([BLK, ET], f32, tag="oacc")
                oacc_b = sp.tile([BLK, ET], bf16, tag="oacc_b")
                for m in range(8):
                    ph = psh.tile([BLK, NS], f32, space="PSUM", tag="ph")
                    for kc in range(8):
                        nc.tensor.matmul(ph[:], miw2_t[:, kc, m * BLK:(m + 1) * BLK],
                                         h[:, kc, :], start=(kc == 0), stop=(kc == 7))
                    h2m = sp.tile([BLK, ET], bf16, tag="h2m")
                    silu_to(h2m[:], ph[:], bmi2_t[:, m:m + 1])
                    if m == 0:
                        nc.vector.tensor_scalar(out=oacc[:], in0=h2m[:],
                                                scalar1=mowf_t[:, 0:1], scalar2=None,
                                                op0=OP.mult)
                    else:
                        nc.vector.scalar_tensor_tensor(
                            out=(oacc_b[:] if m == 7 else oacc[:]), in0=h2m[:],
                            scalar=mowf_t[:, m:m + 1],
                            in1=oacc[:], op0=OP.mult, op1=OP.add)
                nc.tensor.matmul(po[0:1, :], ones_t[:, 0:1], oacc_b[:],
                                 start=True, stop=True)
                ot = sp.tile([1, ET], f32, tag="ot")
                nc.scalar.activation(ot[:], po[0:1, :], AF.Identity,
                                     bias=bmo_t[:, 0:1], scale=1.0)
                nc.sync.dma_start(outd[t], ot[:])

    nc.finalize()
    return nc


def _np_bf16():
    import concourse.mybir as mybir
    return mybir.dt.np(mybir.dt.bfloat16)


def _host_prep(inputs):
    """Shared (replicated) host-side tensors."""
    f = np.float32
    bf = _np_bf16()
    nodes = np.asarray(inputs["nodes"], f)
    W0 = np.asarray(inputs["W0"], f)
    W1 = np.asarray(inputs["W1"], f)
    W2 = np.asarray(inputs["W2"], f)
    ln_g = np.asarray(inputs["ln_g"], f)

    sym = lambda W: 0.5 * (W + W.transpose(1, 0, 2))
    w0f = np.ascontiguousarray((sym(W0) / FAN).reshape(L0 * L0, NS))
    w1f = np.ascontiguousarray((sym(W1) / (FAN * math.sqrt(3.0))).reshape(L1 * L1, NS))
    w2f = np.ascontiguousarray((sym(W2) / (FAN * math.sqrt(5.0))).reshape(L2 * L2, NS))
    miw1 = np.ascontiguousarray(ln_g[:, None] * np.asarray(inputs["mi_w1"], f))

    def colbias(b, nch):
        b = np.asarray(b, f).reshape(nch, BLK)
        return np.ascontiguousarray(b.T)

    cn = np.broadcast_to((np.arange(1, NB + 1, dtype=f) / (2.0 * CUT))[None, :],
                         (BLK, NB)).copy()

    return dict(
        nodesB=nodes.astype(bf),
        w0d=w0f.astype(bf), w1d=w1f.astype(bf), w2d=w2f.astype(bf),
        dfw1d=np.asarray(inputs["df_w1"], f).astype(bf),
        dfw2d=np.asarray(inputs["df_w2"], f).astype(bf),
        miw1d=miw1.astype(bf),
        miw2d=np.asarray(inputs["mi_w2"], f).astype(bf),
        mowd=np.asarray(inputs["mo_w"], f),
        bdf1=colbias(inputs["df_b1"], 8), bdf2=colbias(inputs["df_b2"], 4),
        bmi1=colbias(inputs["mi_b1"], 8), bmi2=colbias(inputs["mi_b2"], 8),
        bmo=np.asarray(inputs["mo_b"], f).reshape(1, 1),
        cnd=cn,
        identd=np.eye(BLK, dtype=f).astype(bf),
        onesd=np.ones((BLK, BLK), f).astype(bf),
    )


def _edge_prep(inputs, core, ntiles):
    """Per-core edge tensors: tiled indices + per-edge radial scalars."""
    f = np.float32
    ec = ntiles * ET
    lo = core * EC
    ei = np.asarray(inputs["edge_index"])
    src = ei[0, lo:lo + ec].astype(np.int32)
    dst = ei[1, lo:lo + ec].astype(np.int32)
    bv = np.asarray(inputs["batch_vec"]).astype(np.int32)
    shift = np.asarray(inputs["edge_shift"], f)[lo:lo + ec]
    pos = np.asarray(inputs["pos"], f)
    cell = np.asarray(inputs["cell"], f)

    bcell = cell[bv[src]]                              # (ec,3,3)
    tvec = np.einsum('ei,eij->ej', shift, bcell)
    radvec = pos[dst] - pos[src] + tvec
    dist = np.sqrt((radvec * radvec).sum(1)) + 1e-6    # (ec,)
    r = 1.0 / dist
    rd = np.stack([r, EMBC * dist], axis=1).astype(f)  # (ec, 2)

    def tile_idx(x):
        return np.ascontiguousarray(x.reshape(ntiles, NBLK, BLK).transpose(0, 2, 1))

    return dict(
        srcidx=tile_idx(src), dstidx=tile_idx(dst),
        rdd=np.ascontiguousarray(
            rd.reshape(ntiles, NBLK, BLK, 2).transpose(0, 2, 1, 3)),
    )


def _run(inputs, mode, ntiles, ncores):
    key = (mode, ntiles, 1)
    if key not in _cache:
        _cache[key] = _build(mode, ntiles)
    nc = _cache[key]
    shared = _host_prep(inputs)
    in_maps = []
    for c in range(ncores):
        m = dict(shared)
        m.update(_edge_prep(inputs, c, ntiles))
        in_maps.append(m)

    if mode == "sim":
        from concourse.bass_interp import CoreSim
        outs = []
        for c in range(ncores):
            sim = CoreSim(nc)
            for k, v in in_maps[c].items():
                sim.tensor(k)[:] = v
            sim.simulate()
            outs.append(np.array(sim.tensor("out")).reshape(-1))
        return np.concatenate(outs).reshape(-1, 1)

    from concourse.bass_utils import run_bass_kernel_spmd
    trace = os.environ.get("EXB_TRACE", "0") == "1"
    res = run_bass_kernel_spmd(nc, in_maps, list(range(ncores)), trace=trace)
    out = np.concatenate([res.results[c]["out"].reshape(-1) for c in range(ncores)])
    if trace:
        _run.last_exec_time_ns = res.exec_time_ns
    return out.reshape(-1, 1)


def kernel(**inputs) -> np.ndarray:
    return _run(inputs, os.environ.get("EXB_MODE", "hw"), EC // ET, NCORES).astype(np.float32)
